# revision 35
# baseline (speedup 1.0000x reference)
"""Causal self-attention (B=2, T=2048, D=1024, 16 heads) on 8 trn2 cores.

Sharding: data-parallel over batch (4 cores per batch element), tensor-parallel
over heads (4 heads per core). Each core computes qkv/attention/proj for its
4 heads and produces a partial [T, D] projection output; the host sums the 4
partials of each batch element.

Host-side prep per core: x[b] transposed to [D, T] (the PE contracts over the
partition dim, so x^T is needed as the streaming operand) and the relevant
w_qkv / w_proj column/row slices, all cast to bf16. The 1/sqrt(d_head) score
scale is folded into w_q and w_k (each gets d_head**-0.25).

v2: software-pipelined emission. Input DMAs are chunk-interleaved (wqk_t,
xT_t pairs) so the qkv matmuls stream behind the loads; S-score blocks are
woven with independent PE work (v/qk/pv/proj quanta) at block granularity so
the PE never drains while ACT chews through the exps; diagonal causal masks
are per-block multiplies on the gpsimd engine; the v_aug ones-columns replace
a full-tile memset.
"""

import numpy as np
import ml_dtypes

import concourse.bass as bass
import concourse.mybir as mybir
import concourse.tile as tile
from concourse import bacc
from concourse.bass_utils import run_bass_kernel_spmd
from concourse.masks import make_identity, make_upper_triangular

B, T, D = 2, 2048, 1024
NH, DH = 16, 64
HPC = 4  # heads per core
NCORES = 8
KT = D // 128  # 8 contraction chunks for qkv matmuls
NT = T // 128  # 16 sequence chunks

BF16 = mybir.dt.bfloat16
F16 = mybir.dt.float16
F32 = mybir.dt.float32
EXP = mybir.ActivationFunctionType.Exp

SLAB = 1024  # tq columns per attention slab
NS = T // SLAB
ND = SLAB // 128  # 128-blocks per slab

TRACE = False
LAST_RESULTS = None
_NC_CACHE = {}
PHASES = []  # (label, first_instruction_number) markers for timeline analysis


def _mark(nc, label):
    PHASES.append((label, int(nc.get_next_instruction_name().split("-")[1])))


def _build_program(loop_n=None):
    nc = bacc.Bacc("TRN2", target_bir_lowering=False, debug=False, num_devices=NCORES)
    xT_d = nc.dram_tensor("xT", [D, T], BF16, kind="ExternalInput").ap()
    wqk_d = nc.dram_tensor("wqk", [D, 2 * HPC * DH], BF16, kind="ExternalInput").ap()
    wv_d = nc.dram_tensor("wv", [D, HPC * DH], BF16, kind="ExternalInput").ap()
    wpr_d = nc.dram_tensor("wpr", [HPC * DH, D], BF16, kind="ExternalInput").ap()
    out_d = nc.dram_tensor("out", [T, D], F16, kind="ExternalOutput").ap()

    with tile.TileContext(nc) as tc:
        if loop_n is None:
            _emit(nc, tc, xT_d, wqk_d, wv_d, wpr_d, out_d)
        else:
            hints = (
                mybir.EngineType.PE,
                mybir.EngineType.Activation,
                mybir.EngineType.DVE,
                mybir.EngineType.SP,
                mybir.EngineType.Pool,
            )
            with tc.For_i(0, loop_n, 1, hint_engines=hints):
                _emit(nc, tc, xT_d, wqk_d, wv_d, wpr_d, out_d)
    nc.compile()
    return nc


def _emit(nc, tc, xT_d, wqk_d, wv_d, wpr_d, out_d):
    with (
        tc.tile_pool(name="big", bufs=1) as big,
        tc.tile_pool(name="pt_pool", bufs=2) as pt_pool,
        tc.tile_pool(name="small", bufs=1) as small,
        tc.tile_pool(name="stage", bufs=3) as stage,
        tc.tile_pool(name="ps_mm", bufs=4, space="PSUM") as ps_mm,
        tc.tile_pool(name="ps_s", bufs=2, space="PSUM") as ps_s,
    ):
        # ---- SBUF tiles ----
        xT_s = big.tile([128, KT, T], BF16)
        wqk_s = big.tile([128, KT, 2 * HPC * DH], BF16)
        wv_s = big.tile([128, KT, HPC * DH], BF16)
        wpr_s = big.tile([128, 2, D], BF16)

        # ---- input DMAs: (wqk_t, xT_t) pairs so qkv matmuls can stream;
        # wv lands just before the final xT chunk so v matmuls can follow ----
        xT_r = xT_d.rearrange("(a p) t -> p a t", p=128)
        wqk_r = wqk_d.rearrange("(a p) n -> p a n", p=128)
        for t in range(KT):
            nc.sync.dma_start(out=wqk_s[:, t, :], in_=wqk_r[:, t, :])
            if t == 0:
                nc.scalar.dma_start(out=xT_s[:, t, :], in_=xT_r[:, t, :])
            else:
                nc.sync.dma_start(out=xT_s[:, t, :], in_=xT_r[:, t, :])
        nc.sync.dma_start(out=wv_s, in_=wv_d.rearrange("(a p) n -> p a n", p=128))
        nc.sync.dma_start(out=wpr_s, in_=wpr_d.rearrange("(a p) n -> p a n", p=128))

        ident = small.tile([128, 128], BF16)
        make_identity(nc, ident)
        # Dummy exp so walrus's ACT table load (~2.7us) happens during the
        # input-DMA ramp instead of at the first real exp on the critical path.
        warm = small.tile([128, 1], F32)
        nc.vector.memset(warm, 0.0)
        nc.scalar.activation(warm, warm, EXP)
        # gemask[p, f] = 1.0 where f >= p: the valid (tq >= tk) part of the
        # diagonal 128x128 block of S^T.
        gemask = small.tile([128, 128], BF16)
        make_upper_triangular(nc, gemask, val=1.0, diag=True)

        # q^T / k^T in [d, T] layout: tile jt holds heads 2*jt (parts 0:64)
        # and 2*jt+1 (parts 64:128).
        qT_s = big.tile([128, 2, T], BF16)
        kT_s = big.tile([128, 2, T], BF16)
        # v in natural [tk, d] layout plus a ones-column per head for rowsums
        v_aug = big.tile([128, NT, 66 * HPC], BF16)
        for h in range(HPC):
            nc.gpsimd.memset(v_aug[:, :, 66 * h + DH : 66 * h + DH + 1], 1.0)
        y_all = big.tile([128, NT, HPC * DH], BF16)
        yT_s = big.tile([128, 2, T], BF16)

        pt = {}  # (h, s) -> pt slab tile

        def qk_q(m, n):
            # one 512-col stripe of q^T/k^T rows [128m : 128m+128]
            ps = ps_mm.tile([128, 512], F32, tag="mm")
            for t in range(KT):
                nc.tensor.matmul(
                    ps,
                    lhsT=wqk_s[:, t, 128 * m : 128 * (m + 1)],
                    rhs=xT_s[:, t, 512 * n : 512 * (n + 1)],
                    start=(t == 0),
                    stop=(t == KT - 1),
                )
            dst = qT_s if m < 2 else kT_s
            nc.vector.tensor_copy(dst[:, m % 2, 512 * n : 512 * (n + 1)], ps)

        def qk_stream(stripes):
            # chunk-major accumulation over 8 stripes (4 singles in ps_mm,
            # 2 pairs sharing [128,1024] ps_s tiles) so the PE rides the
            # input-DMA stream instead of idling per chunk
            singles, pairs = stripes[:4], stripes[4:]
            assert len(pairs) % 2 == 0
            ps_sg = [
                ps_mm.tile([128, 512], F32, tag="mm", name=f"qs{i}")
                for i in range(len(singles))
            ]
            ps_pr = [
                ps_s.tile([128, SLAB], F32, tag="s", name=f"qp{i}")
                for i in range(len(pairs) // 2)
            ]
            units = [(m, n, ps_sg[i][:, :]) for i, (m, n) in enumerate(singles)]
            for i, (m, n) in enumerate(pairs):
                units.append((m, n, ps_pr[i // 2][:, 512 * (i % 2) : 512 * (i % 2 + 1)]))
            for t in range(KT):
                for m, n, ps in units:
                    nc.tensor.matmul(
                        ps,
                        lhsT=wqk_s[:, t, 128 * m : 128 * (m + 1)],
                        rhs=xT_s[:, t, 512 * n : 512 * (n + 1)],
                        start=(t == 0),
                        stop=(t == KT - 1),
                        skip_group_check=True,
                    )
            for m, n, ps in units:
                dst = qT_s if m < 2 else kT_s
                nc.vector.tensor_copy(dst[:, m % 2, 512 * n : 512 * (n + 1)], ps)

        def v_q(j):
            # v rows [128j : 128j+128] natural, scattered into v_aug
            ps = ps_mm.tile([128, HPC * DH], F32, tag="mm")
            for t in range(KT):
                nc.tensor.matmul(
                    ps,
                    lhsT=xT_s[:, t, 128 * j : 128 * (j + 1)],
                    rhs=wv_s[:, t, :],
                    start=(t == 0),
                    stop=(t == KT - 1),
                )
            nc.vector.tensor_copy(
                v_aug[:, j, :].rearrange("p (h c) -> p h c", c=66)[:, :, 0:DH],
                ps.rearrange("p (h c) -> p h c", c=DH),
            )

        def s_q(h, s, i):
            # S^T block (tk chunk i) of slab s: matmul + exp (+ diag mask)
            jt, base = h // 2, 64 * (h % 2)
            qT_h = qT_s[base : base + 64, jt, :]
            kT_h = kT_s[base : base + 64, jt, :]
            c_lo = max(SLAB * s, 128 * i)
            w = SLAB * (s + 1) - c_lo
            ptile = pt[(h, s)]
            ps = ps_s.tile([128, SLAB], F32, tag="s")
            for c in range(0, w, 512):
                cw = min(512, w - c)
                nc.tensor.matmul(
                    ps[:, c : c + cw],
                    lhsT=kT_h[:, 128 * i : 128 * (i + 1)],
                    rhs=qT_h[:, c_lo + c : c_lo + c + cw],
                    start=True,
                    stop=True,
                )
            off = c_lo - SLAB * s
            nc.scalar.activation(ptile[:, i, off : off + w], ps[:, 0:w], EXP)
            if i >= ND * s and i < ND * (s + 1):
                # diagonal block: zero the tq < tk half on the gpsimd engine
                r = i - ND * s
                dv = ptile[:, i, 128 * r : 128 * (r + 1)]
                nc.gpsimd.tensor_mul(dv, dv, gemask)

        def s_alloc(h, s):
            nblk = ND * (s + 1)  # slab s touches tk blocks 0 .. ND*(s+1)-1
            pt[(h, s)] = pt_pool.tile(
                [128, nblk, SLAB], BF16, tag=f"pt{s}", name=f"pt{h}{s}"
            )

        def pv_q(h, s, jl):
            # y[tq block j, head h] = sum_tk P~[tq, tk] v[tk, :], col 64 = rowsum
            j = ND * s + jl
            ptile = pt[(h, s)]
            ps = ps_mm.tile([128, 68], F32, tag="mm")
            for i in range(j + 1):
                nc.tensor.matmul(
                    ps[:, 0:65],
                    lhsT=ptile[:, i, 128 * jl : 128 * (jl + 1)],
                    rhs=v_aug[:, i, 66 * h : 66 * h + 65],
                    start=(i == 0),
                    stop=(i == j),
                )
            rinv = stage.tile([128, 1], F32, tag="rinv")
            nc.vector.reciprocal(rinv, ps[:, DH : DH + 1])
            nc.vector.tensor_scalar_mul(
                y_all[:, j, DH * h : DH * (h + 1)], ps[:, 0:DH], rinv
            )

        def trans_q(j, act_copy=False):
            # y^T rows for block j via PE transpose
            for dm in range(2):
                pst = ps_mm.tile([128, 128], BF16, tag="mm")
                nc.tensor.transpose(
                    pst, y_all[:, j, 128 * dm : 128 * (dm + 1)], ident
                )
                if act_copy:
                    nc.scalar.copy(yT_s[:, dm, 128 * j : 128 * (j + 1)], pst)
                else:
                    nc.vector.tensor_copy(yT_s[:, dm, 128 * j : 128 * (j + 1)], pst)

        def proj_q(j, act_copy=False, split_copy=False):
            # out rows [128j : 128j+128] = y[j] @ wpr (fp16 partial)
            for n in range(2):
                ps = ps_mm.tile([128, 512], F32, tag="mm")
                for dm in range(2):
                    nc.tensor.matmul(
                        ps,
                        lhsT=yT_s[:, dm, 128 * j : 128 * (j + 1)],
                        rhs=wpr_s[:, dm, 512 * n : 512 * (n + 1)],
                        start=(dm == 0),
                        stop=(dm == 1),
                    )
                ost = stage.tile([128, 512], F16, tag="ost")
                if split_copy:
                    # halves on both engines so the drain runs in parallel
                    nc.vector.tensor_copy(ost[:, 0:256], ps[:, 0:256])
                    nc.scalar.copy(ost[:, 256:512], ps[:, 256:512])
                elif act_copy:
                    nc.scalar.copy(ost, ps)
                else:
                    nc.vector.tensor_copy(ost, ps)
                nc.sync.dma_start(
                    out=out_d[128 * j : 128 * (j + 1), 512 * n : 512 * (n + 1)],
                    in_=ost,
                )

        def M(label):
            _mark(nc, label)

        # ---- emission schedule: S blocks woven with independent PE work ----
        M("load")
        qk_stream(
            [(2, 0), (0, 0), (0, 1), (2, 1), (0, 2), (0, 3), (2, 2), (2, 3)]
        )
        M("s00")
        s_alloc(0, 0)
        s_q(0, 0, 0); s_q(0, 0, 1)
        v_q(0)
        s_q(0, 0, 2); s_q(0, 0, 3)
        v_q(1)
        s_q(0, 0, 4); s_q(0, 0, 5)
        v_q(2)
        s_q(0, 0, 6); s_q(0, 0, 7)
        v_q(3)
        M("s10")
        s_alloc(1, 0)
        s_q(1, 0, 0); s_q(1, 0, 1)
        v_q(4)
        s_q(1, 0, 2); s_q(1, 0, 3)
        v_q(5)
        s_q(1, 0, 4); s_q(1, 0, 5)
        v_q(6)
        s_q(1, 0, 6); s_q(1, 0, 7)
        v_q(7)
        M("s01")
        s_alloc(0, 1)
        s_q(0, 1, 0); v_q(8)
        s_q(0, 1, 1); v_q(9)
        s_q(0, 1, 2); v_q(10)
        s_q(0, 1, 3); v_q(11)
        s_q(0, 1, 4); pv_q(0, 0, 0); pv_q(0, 0, 1); pv_q(0, 0, 2)
        s_q(0, 1, 5); pv_q(0, 0, 3); pv_q(0, 0, 4)
        s_q(0, 1, 6); pv_q(0, 0, 5); pv_q(0, 0, 6)
        s_q(0, 1, 7); pv_q(0, 0, 7)
        s_q(0, 1, 8); pv_q(1, 0, 0); pv_q(1, 0, 1); pv_q(1, 0, 2)
        s_q(0, 1, 9); pv_q(1, 0, 3); pv_q(1, 0, 4)
        s_q(0, 1, 10); pv_q(1, 0, 5); pv_q(1, 0, 6)
        s_q(0, 1, 11); pv_q(1, 0, 7)
        s_q(0, 1, 12); qk_q(1, 0)
        s_q(0, 1, 13); qk_q(1, 1)
        s_q(0, 1, 14); qk_q(1, 2)
        s_q(0, 1, 15); qk_q(1, 3)
        M("s11")
        s_alloc(1, 1)
        s_q(1, 1, 0); s_q(1, 1, 1); qk_q(3, 0)
        s_q(1, 1, 2); s_q(1, 1, 3); qk_q(3, 1)
        s_q(1, 1, 4); s_q(1, 1, 5); qk_q(3, 2)
        s_q(1, 1, 6); s_q(1, 1, 7); qk_q(3, 3)
        s_q(1, 1, 8); v_q(12)
        s_q(1, 1, 9); v_q(13)
        s_q(1, 1, 10)
        s_q(1, 1, 11)
        s_q(1, 1, 12); pv_q(0, 1, 0); pv_q(0, 1, 1)
        s_q(1, 1, 13); pv_q(0, 1, 2); pv_q(0, 1, 3)
        s_q(1, 1, 14); v_q(14)
        s_q(1, 1, 15); v_q(15)
        M("s20")
        s_alloc(2, 0)
        s_q(2, 0, 0); pv_q(0, 1, 4); pv_q(0, 1, 5)
        s_q(2, 0, 1); pv_q(0, 1, 6); pv_q(0, 1, 7)
        s_q(2, 0, 2); pv_q(1, 1, 0)
        s_q(2, 0, 3); pv_q(1, 1, 1)
        s_q(2, 0, 4); pv_q(1, 1, 2)
        s_q(2, 0, 5); pv_q(1, 1, 3)
        s_q(2, 0, 6); pv_q(1, 1, 4)
        s_q(2, 0, 7); pv_q(1, 1, 5)
        M("s30")
        s_alloc(3, 0)
        s_q(3, 0, 0); pv_q(1, 1, 6); pv_q(1, 1, 7)
        s_q(3, 0, 1); pv_q(2, 0, 0); pv_q(2, 0, 1)
        s_q(3, 0, 2); pv_q(2, 0, 2)
        s_q(3, 0, 3); pv_q(2, 0, 3)
        s_q(3, 0, 4); pv_q(2, 0, 4)
        s_q(3, 0, 5); pv_q(2, 0, 5)
        s_q(3, 0, 6); pv_q(2, 0, 6)
        s_q(3, 0, 7); pv_q(2, 0, 7)
        M("s21")
        s_alloc(2, 1)
        s_q(2, 1, 0); pv_q(3, 0, 0); pv_q(3, 0, 1)
        s_q(2, 1, 1); pv_q(3, 0, 2); pv_q(3, 0, 3)
        s_q(2, 1, 2); pv_q(3, 0, 4); pv_q(3, 0, 5)
        s_q(2, 1, 3); pv_q(3, 0, 6); pv_q(3, 0, 7)
        s_q(2, 1, 4); trans_q(0); trans_q(1)
        s_q(2, 1, 5); trans_q(2); trans_q(3)
        s_q(2, 1, 6); proj_q(0)
        s_q(2, 1, 7); proj_q(1)
        s_q(2, 1, 8); trans_q(4); trans_q(5)
        s_q(2, 1, 9); proj_q(2)
        s_q(2, 1, 10); trans_q(6); trans_q(7)
        s_q(2, 1, 11); proj_q(3)
        s_q(2, 1, 12); proj_q(4)
        s_q(2, 1, 13)
        s_q(2, 1, 14)
        s_q(2, 1, 15)
        M("s31")
        s_alloc(3, 1)
        s_q(3, 1, 0); pv_q(2, 1, 0); pv_q(2, 1, 1)
        s_q(3, 1, 1); pv_q(2, 1, 2); pv_q(2, 1, 3)
        s_q(3, 1, 2); pv_q(2, 1, 4)
        s_q(3, 1, 3); pv_q(2, 1, 5)
        s_q(3, 1, 4); pv_q(2, 1, 6)
        s_q(3, 1, 5); pv_q(2, 1, 7)
        s_q(3, 1, 6); proj_q(5)
        s_q(3, 1, 7)
        s_q(3, 1, 8); proj_q(6)
        s_q(3, 1, 9)
        s_q(3, 1, 10); proj_q(7)
        s_q(3, 1, 11)
        M("tail")
        s_q(3, 1, 12); pv_q(3, 1, 0)
        s_q(3, 1, 13); pv_q(3, 1, 1); trans_q(8)
        s_q(3, 1, 14); pv_q(3, 1, 2); trans_q(9); proj_q(8)
        s_q(3, 1, 15); pv_q(3, 1, 3); trans_q(10); proj_q(9)
        pv_q(3, 1, 4); trans_q(11); proj_q(10)
        pv_q(3, 1, 5); trans_q(12, act_copy=True); proj_q(11)
        pv_q(3, 1, 6); trans_q(13, act_copy=True); proj_q(12, act_copy=True)
        pv_q(3, 1, 7); trans_q(14, act_copy=True); trans_q(15, act_copy=True)
        proj_q(13, act_copy=True)
        proj_q(14, act_copy=True)
        proj_q(15, split_copy=True)
        M("end")


def _get_nc():
    if "nc" not in _NC_CACHE:
        _NC_CACHE["nc"] = _build_program()
    return _NC_CACHE["nc"]


def make_in_maps(x, w_qkv, w_proj):
    bf16 = ml_dtypes.bfloat16
    scale = np.float32(DH**-0.25)
    x = np.asarray(x, dtype=np.float32)
    w_qkv = np.asarray(w_qkv, dtype=np.float32)
    w_proj = np.asarray(w_proj, dtype=np.float32)
    xT_b = [np.ascontiguousarray(x[b].T).astype(bf16) for b in range(B)]
    in_maps = []
    for c in range(NCORES):
        b, g = c // HPC, c % HPC
        cs = slice(g * HPC * DH, (g + 1) * HPC * DH)  # 256 cols of this head group
        wq = w_qkv[:, 0 * D : 1 * D][:, cs] * scale
        wk = w_qkv[:, 1 * D : 2 * D][:, cs] * scale
        in_maps.append(
            {
                "xT": xT_b[b],
                "wqk": np.concatenate([wq, wk], axis=1).astype(bf16),
                "wv": np.ascontiguousarray(w_qkv[:, 2 * D : 3 * D][:, cs]).astype(bf16),
                "wpr": np.ascontiguousarray(w_proj[cs, :]).astype(bf16),
            }
        )
    return in_maps


def kernel(x, w_qkv, w_proj):
    global LAST_RESULTS
    nc = _get_nc()
    in_maps = make_in_maps(x, w_qkv, w_proj)
    res = run_bass_kernel_spmd(nc, in_maps, list(range(NCORES)), trace=TRACE)
    LAST_RESULTS = res
    parts = [np.asarray(res.results[c]["out"], dtype=np.float32) for c in range(NCORES)]
    out = np.stack([sum(parts[b * HPC : (b + 1) * HPC]) for b in range(B)], axis=0)
    return out.astype(np.float32)
